# revision 9
# baseline (speedup 1.0000x reference)
"""BFLinear (block-floating-point quantized linear) Trainium2 kernel.

Computes: out = bf_quant(bf_quant(x) @ bf_quant(W).T + 2*b)
where bf_quant quantizes groups of 32 along the last axis to a shared
power-of-two exponent with 8 mantissa bits (values = int8 * 2^(e-7)).

Distribution over 8 NeuronCores:
  - batch dim of x sharded 8 ways (1024 rows/core)
  - W quantization split by output rows (512 rows/core); each core
    PE-transposes its quantized slab to [in, out_slice] layout and the
    bf16 slabs are AllGathered (split into two k-halves so remote data
    streams in early)
  - all DMA is contiguous (>=1KB lines); the quantized transposed x
    (xqT) stays resident in SBUF
  - matmul runs in bf16 (exact products, fp32 PSUM accumulation; the
    dot products are exactly representable in fp32 for this data)
  - bias (x2) is added during the PSUM drain, fused with output quant

Quantization math (all exact, matching jnp semantics, verified on HW):
  m     = max |x| over each group of 32          (abs-max reduce)
  scale = 2^(floor(log2 m) - 7)                  (exponent-field bit math)
  inv   = 1/scale                                (bit math, exact)
  r     = rne_round(clamp(x*inv)) via +C trick with C = 1.5*2^23
  q     = (r - C) * scale
"""

import numpy as np

# full-problem dimensions (hardcoded per harness contract)
B_FULL = 8192
IN_FULL = 4096
OUT_FULL = 4096
NCORES = 8

P = 128
SZ = 32
NB = 512  # output column block (= w_sl for 8 cores)
C_RND = float(3 * 2**22)  # 1.5*2^23: v+C stays in [2^23, 2^24) -> RNE to ints


def build_nc(b_sh=B_FULL // NCORES, in_dim=IN_FULL, out_dim=OUT_FULL,
             ncores=NCORES, for_timeline=False):
    """Build the SPMD Bass program (identical on every core; data differs)."""
    import concourse.mybir as mybir
    import concourse.tile as tile
    from concourse import bacc

    F32 = mybir.dt.float32
    BF16 = mybir.dt.bfloat16
    I32 = mybir.dt.int32
    ALU = mybir.AluOpType
    AX = mybir.AxisListType
    AF = mybir.ActivationFunctionType

    w_sl = out_dim // ncores          # W rows quantized on this core
    kc = in_dim // P                  # 128-wide contraction chunks
    kh = kc // 2                      # AllGather k-split point
    n_xt = b_sh // P                  # x row tiles
    n_wt = w_sl // P                  # W row tiles
    n_jb = out_dim // NB              # output column blocks (== ncores)
    assert w_sl == NB and n_jb == ncores
    assert in_dim % (4 * P) == 0 and b_sh % P == 0

    nc = bacc.Bacc("TRN2", target_bir_lowering=False, debug=False,
                   num_devices=ncores)

    x_sh = nc.dram_tensor("x_sh", [b_sh, in_dim], F32, kind="ExternalInput")
    w_sl_t = nc.dram_tensor("w_sl", [w_sl, in_dim], F32, kind="ExternalInput")
    b2_rep = nc.dram_tensor("b2_rep", [P, out_dim], F32, kind="ExternalInput")
    ident_in = nc.dram_tensor("ident", [P, P], BF16, kind="ExternalInput")
    out_sh = nc.dram_tensor("out_sh", [b_sh, out_dim], F32,
                            kind="ExternalOutput")

    # local quantized+transposed W slab, and the gathered halves
    wqt_loc = nc.dram_tensor("wqt_loc", [in_dim, NB], BF16)
    wq_ag0 = nc.dram_tensor("wq_ag0", [ncores * kh * P, NB], BF16,
                            addr_space="Shared")
    wq_ag1 = nc.dram_tensor("wq_ag1", [ncores * kh * P, NB], BF16,
                            addr_space="Shared")

    with tile.TileContext(nc) as tc:
        from contextlib import ExitStack
        with ExitStack() as ctx:
            qpool = ctx.enter_context(tc.tile_pool(name="qpool", bufs=2))
            spool = ctx.enter_context(tc.tile_pool(name="spool", bufs=2))
            big = ctx.enter_context(tc.tile_pool(name="big", bufs=1))
            wpool = ctx.enter_context(tc.tile_pool(name="wpool", bufs=4))
            opool = ctx.enter_context(tc.tile_pool(name="opool", bufs=4))
            pt_pool = ctx.enter_context(
                tc.tile_pool(name="ptp", bufs=2, space="PSUM"))
            mm_pool = ctx.enter_context(
                tc.tile_pool(name="mmp", bufs=1, space="PSUM"))

            ident = big.tile([P, P], BF16, tag="ident")
            nc.sync.dma_start(ident[:], ident_in.ap())
            b2_sb = big.tile([P, out_dim], F32, tag="b2_sb")
            nc.sync.dma_start(b2_sb[:], b2_rep.ap())

            def quant_tile(eng, xt, rows, width, out_dtype, qp, sp, tagp):
                """Quantize an SBUF-resident [rows, width] f32 tile in place
                (xt is clobbered); returns q tile with dtype out_dtype.
                Only the big mult runs on `eng` (Pool-capable); reduce/bits/
                clamp/stt are DVE-only ops; RNE rounding on scalar."""
                g = width // SZ
                x3 = xt.rearrange("p (g s) -> p g s", s=SZ)
                m = sp.tile([rows, g], F32, tag=f"{tagp}_m")
                # free-axis reduce is DVE-only; all else runs on `eng`
                nc.vector.tensor_reduce(m[:], x3, axis=AX.X, op=ALU.max,
                                        apply_absolute_value=True)
                scale = sp.tile([rows, g], F32, tag=f"{tagp}_scale")
                # scale_bits = (m_bits & 0x7F800000) - (7 << 23)
                # (int32 bitwise ops are DVE-only; these tiles are small)
                nc.vector.tensor_scalar(
                    scale[:].bitcast(I32), m[:].bitcast(I32),
                    0x7F800000, None, op0=ALU.bitwise_and)
                nc.vector.tensor_scalar(
                    scale[:].bitcast(I32), scale[:].bitcast(I32),
                    7 << 23, None, op0=ALU.subtract)
                inv = sp.tile([rows, g], F32, tag=f"{tagp}_inv")
                # inv_bits = (254<<23) - scale_bits
                nc.vector.tensor_scalar(
                    inv[:].bitcast(I32), scale[:].bitcast(I32),
                    -1, None, op0=ALU.bitwise_xor)
                nc.vector.tensor_scalar(
                    inv[:].bitcast(I32), inv[:].bitcast(I32),
                    (254 << 23) + 1, None, op0=ALU.add)
                # v = x * inv (exact power-of-two scaling)
                eng.tensor_tensor(
                    x3, x3, inv[:, :, None].to_broadcast([rows, g, SZ]),
                    ALU.mult)
                # pre-round clamp; round(clip(v,-128.25,127.25)) ==
                # clip(round(v), -128, 127) since round is monotonic
                nc.vector.tensor_scalar(
                    xt, xt, -128.25, 127.25, op0=ALU.max, op1=ALU.min)
                # +C rounds to nearest-even integer; scalar engine
                nc.scalar.activation(xt, xt, AF.Copy, bias=C_RND, scale=1.0)
                # q = (r - C) * scale
                q = qp.tile([rows, width], out_dtype, tag=f"{tagp}_q")
                nc.vector.scalar_tensor_tensor(
                    q[:].rearrange("p (g s) -> p g s", s=SZ),
                    xt.rearrange("p (g s) -> p g s", s=SZ),
                    C_RND,
                    scale[:, :, None].to_broadcast([rows, g, SZ]),
                    op0=ALU.subtract, op1=ALU.mult)
                return q

            def transpose_tile(q, dst_col, dst, tag):
                """PE-transpose q [P, in_dim] bf16 into dst[:, k, dst_col
                block]; 4 chunk-transposes per PSUM bank, one copy each.
                Copies read PSUM, so they must run on DVE (not gpsimd)."""
                for kq in range(kc // 4):
                    tp = pt_pool.tile([P, 4, P], BF16, tag=f"{tag}_tp")
                    for t in range(4):
                        k = kq * 4 + t
                        nc.tensor.transpose(tp[:, t, :], q[:, k * P:(k + 1) * P],
                                            ident[:])
                    nc.scalar.copy(
                        dst[:, kq * 4:(kq + 1) * 4, dst_col:dst_col + P],
                        tp[:])

            engs = None  # set after engines exist

            # ---- W stage: quantize + transpose local slab ------------------
            wqT_sb = big.tile([P, kc, NB], BF16, tag="wqT_sb")
            w_engs = [1, 1, 1, 1] * ((n_wt + 3) // 4)
            for i in range(n_wt):
                ei = w_engs[i]
                eng = nc.vector if ei == 0 else nc.gpsimd
                en = "v" if ei == 0 else "g"
                wt = qpool.tile([P, in_dim], F32, tag=f"q_{en}")
                nc.sync.dma_start(wt[:], w_sl_t.ap()[i * P:(i + 1) * P, :])
                q = quant_tile(eng, wt[:], P, in_dim, BF16, qpool, spool,
                               f"q_{en}")
                transpose_tile(q[:], i * P, wqT_sb, f"t_{en}")

            # write the two k-halves and AllGather them. The dest AP must
            # iterate (p, k, o) like the source: row of wqt_loc = k*P + p.
            nc.gpsimd.dma_start(
                wqt_loc.ap()[0:kh * P, :].rearrange("(k p) o -> p k o", p=P),
                wqT_sb[:, 0:kh, :])
            nc.gpsimd.dma_start(
                wqt_loc.ap()[kh * P:kc * P, :].rearrange("(k p) o -> p k o",
                                                         p=P),
                wqT_sb[:, kh:kc, :])
            if for_timeline or ncores == 1:
                nc.gpsimd.dma_start(wq_ag0.ap()[0:kh * P, :],
                                    wqt_loc.ap()[0:kh * P, :])
                nc.gpsimd.dma_start(wq_ag1.ap()[0:kh * P, :],
                                    wqt_loc.ap()[kh * P:kc * P, :])
            else:
                nc.gpsimd.collective_compute(
                    "AllGather", ALU.bypass,
                    replica_groups=[list(range(ncores))],
                    ins=[wqt_loc.ap()[0:kh * P, :].opt()],
                    outs=[wq_ag0.ap().opt()])
                nc.gpsimd.collective_compute(
                    "AllGather", ALU.bypass,
                    replica_groups=[list(range(ncores))],
                    ins=[wqt_loc.ap()[kh * P:kc * P, :].opt()],
                    outs=[wq_ag1.ap().opt()])

            # ---- x stage: quantize + transpose into resident xqT -----------
            xqT = big.tile([P, kc, b_sh], BF16, tag="xqT")
            x_engs = [1, 1, 1, 1, 1, 1, 1, 1] * ((n_xt + 7) // 8)
            for i in range(n_xt):
                ei = x_engs[i]
                eng = nc.vector if ei == 0 else nc.gpsimd
                en = "v" if ei == 0 else "g"
                xt = qpool.tile([P, in_dim], F32, tag=f"q_{en}")
                nc.sync.dma_start(xt[:], x_sh.ap()[i * P:(i + 1) * P, :])
                q = quant_tile(eng, xt[:], P, in_dim, BF16, qpool, spool,
                               f"q_{en}")
                transpose_tile(q[:], i * P, xqT, f"t_{en}")

            # ---- matmul + drain --------------------------------------------
            def drain(eng, en, ps, bb, j):
                s = opool.tile([P, NB], F32, tag=f"ds_{en}")
                # psum read must be on DVE (gpsimd cannot access PSUM)
                nc.vector.tensor_tensor(s[:], ps[:],
                                        b2_sb[:, j * NB:(j + 1) * NB], ALU.add)
                oq = quant_tile(eng, s[:], P, NB, F32, opool, opool,
                                f"o_{en}")
                nc.gpsimd.dma_start(
                    out_sh.ap()[bb * P:(bb + 1) * P, j * NB:(j + 1) * NB],
                    oq[:])

            GRP = 4
            dcount = 0
            for j in range(n_jb):
                for g0 in range(0, n_xt, GRP):
                    bbs = list(range(g0, min(g0 + GRP, n_xt)))
                    ps = [mm_pool.tile([P, NB], F32, tag=f"mm{i}",
                                       name=f"ps_{j}_{g0}_{i}")
                          for i in range(len(bbs))]
                    for k in range(kc):
                        wqt = wpool.tile([P, NB], BF16, tag="wqt")
                        if k < kh:
                            src = wq_ag0.ap()[j * kh * P + k * P:
                                              j * kh * P + (k + 1) * P, :]
                        else:
                            src = wq_ag1.ap()[j * kh * P + (k - kh) * P:
                                              j * kh * P + (k - kh + 1) * P, :]
                        nc.sync.dma_start(wqt[:], src)
                        for i, bb in enumerate(bbs):
                            nc.tensor.matmul(
                                ps[i][:],
                                lhsT=xqT[:, k, bb * P:(bb + 1) * P],
                                rhs=wqt[:],
                                start=(k == 0), stop=(k == kc - 1),
                                skip_group_check=True)
                    for i, bb in enumerate(bbs):
                        eng = nc.vector if dcount % 2 == 0 else nc.gpsimd
                        en = "v" if dcount % 2 == 0 else "g"
                        dcount += 1
                        drain(eng, en, ps[i], bb, j)

    nc.compile()
    return nc


_NC_CACHE = {}


def _get_nc(key=(B_FULL // NCORES, IN_FULL, OUT_FULL, NCORES)):
    if key not in _NC_CACHE:
        _NC_CACHE[key] = build_nc(*key)
    return _NC_CACHE[key]


def make_in_maps(x, W, b, ncores=NCORES):
    import ml_dtypes
    b_sh = x.shape[0] // ncores
    w_sl = W.shape[0] // ncores
    out_dim = W.shape[0]
    b2 = (2.0 * np.asarray(b, np.float32)).astype(np.float32)
    b2_rep = np.ascontiguousarray(
        np.broadcast_to(b2.reshape(1, out_dim), (P, out_dim)))
    ident = np.eye(P, dtype=ml_dtypes.bfloat16)
    return [
        {
            "x_sh": np.ascontiguousarray(x[c * b_sh:(c + 1) * b_sh]),
            "w_sl": np.ascontiguousarray(W[c * w_sl:(c + 1) * w_sl]),
            "b2_rep": b2_rep,
            "ident": ident,
        }
        for c in range(ncores)
    ]


def kernel(x, W, b):
    from concourse.bass_utils import run_bass_kernel_spmd

    x = np.asarray(x, np.float32)
    W = np.asarray(W, np.float32)
    b = np.asarray(b, np.float32)
    nc = _get_nc()
    in_maps = make_in_maps(x, W, b)
    res = run_bass_kernel_spmd(nc, in_maps, core_ids=list(range(NCORES)))
    return np.concatenate([res.results[c]["out_sh"] for c in range(NCORES)],
                          axis=0)


# revision 13
# speedup vs baseline: 1.1507x; 1.1507x over previous
"""BFLinear (block-floating-point quantized linear) Trainium2 kernel.

Computes: out = bf_quant(bf_quant(x) @ bf_quant(W).T + 2*b)
where bf_quant quantizes groups of 32 along the last axis to a shared
power-of-two exponent with 8 mantissa bits (values = int8 * 2^(e-7)).

Distribution over 8 NeuronCores:
  - batch dim of x sharded 8 ways (1024 rows/core)
  - W quantization split by output rows (512 rows/core); the quantized
    slab is transposed via XBAR DMA-transpose reads (bit-exact for bf16)
    into [in, out_slice] layout and AllGathered once
  - the transposed quantized x (xqT) is resident in SBUF; all matmul-
    phase weight loads are contiguous 128KB reads
  - matmul runs in bf16 (exact products, fp32 PSUM accumulation; the
    dot products are exactly representable in fp32 for this data)
  - the batch is processed in two half-passes so the second half's
    quantization overlaps the first half's matmuls
  - PE warmup: the tensor engine only reaches 2.4GHz after ~3us of
    gapless execution, so dummy matmuls keep it hot before the first
    real chain and the matmul stream is kept continuous

Quantization math (all exact, matching jnp semantics, verified on HW):
  m     = max |x| over each group of 32          (abs-max reduce)
  scale = 2^(floor(log2 m) - 7)                  (exponent-field bit math)
  inv   = 1/scale                                (bit math, exact)
  r     = rne_round(clamp(x*inv)) via +C trick with C = 1.5*2^23
  q     = (r - C) * scale
"""

import numpy as np

# full-problem dimensions (hardcoded per harness contract)
B_FULL = 8192
IN_FULL = 4096
OUT_FULL = 4096
NCORES = 8

P = 128
SZ = 32
NB = 512  # output column block (= w_sl for 8 cores)
C_RND = float(3 * 2**22)  # 1.5*2^23: v+C stays in [2^23, 2^24) -> RNE to ints


def build_nc(b_sh=B_FULL // NCORES, in_dim=IN_FULL, out_dim=OUT_FULL,
             ncores=NCORES, for_timeline=False, dummy_n=256):
    """Build the SPMD Bass program (identical on every core; data differs)."""
    import concourse.mybir as mybir
    import concourse.tile as tile
    from concourse import bacc

    F32 = mybir.dt.float32
    BF16 = mybir.dt.bfloat16
    I32 = mybir.dt.int32
    ALU = mybir.AluOpType
    AX = mybir.AxisListType
    AF = mybir.ActivationFunctionType

    w_sl = out_dim // ncores          # W rows quantized on this core
    kc = in_dim // P                  # 128-wide contraction chunks
    n_xt = b_sh // P                  # x row tiles
    n_wt = w_sl // P                  # W row tiles
    n_jb = out_dim // NB              # output column blocks (== ncores)
    assert w_sl == NB and n_jb == ncores
    assert in_dim % P == 0 and b_sh % (4 * P) == 0 and n_xt % 4 == 0
    nh = n_xt // 2                    # x tiles per half-pass
    bh = b_sh // 2                    # rows per half-pass

    nc = bacc.Bacc("TRN2", target_bir_lowering=False, debug=False,
                   num_devices=ncores)

    x_sh = nc.dram_tensor("x_sh", [b_sh, in_dim], F32, kind="ExternalInput")
    w_sl_t = nc.dram_tensor("w_sl", [w_sl, in_dim], F32, kind="ExternalInput")
    b2_rep = nc.dram_tensor("b2_rep", [P, out_dim], F32, kind="ExternalInput")
    out_sh = nc.dram_tensor("out_sh", [b_sh, out_dim], F32,
                            kind="ExternalOutput")

    wq_dram = nc.dram_tensor("wq_dram", [w_sl, in_dim], BF16)
    xq_dram = nc.dram_tensor("xq_dram", [b_sh, in_dim], BF16)
    wqt_loc = nc.dram_tensor("wqt_loc", [in_dim, NB], BF16)
    wq_ag = nc.dram_tensor("wq_ag", [ncores * in_dim, NB], BF16,
                           addr_space="Shared")

    with tile.TileContext(nc) as tc:
        from contextlib import ExitStack
        with ExitStack() as ctx:
            qpool = ctx.enter_context(tc.tile_pool(name="qpool", bufs=2))
            spool = ctx.enter_context(tc.tile_pool(name="spool", bufs=2))
            big = ctx.enter_context(tc.tile_pool(name="big", bufs=1))
            wpool = ctx.enter_context(tc.tile_pool(name="wpool", bufs=6))
            opool = ctx.enter_context(tc.tile_pool(name="opool", bufs=4))
            mm_pool = ctx.enter_context(
                tc.tile_pool(name="mmp", bufs=2, space="PSUM"))

            b2_sb = big.tile([P, out_dim], F32, tag="b2_sb")
            nc.scalar.dma_start(b2_sb[:], b2_rep.ap())

            # ---- quant phases (phase-split for software pipelining) --------
            # reduce/bits/clamp/stt are DVE-only ops; the big mult runs on
            # gpsimd (Pool); RNE rounding on the scalar engine.
            def q_load(src, row, tag):
                xt = qpool.tile([P, in_dim], F32, tag=f"{tag}_xt")
                nc.scalar.dma_start(xt[:], src.ap()[row:row + P, :])
                return xt

            def q_scales(xt, tag):
                g = in_dim // SZ
                x3 = xt.rearrange("p (g s) -> p g s", s=SZ)
                m = spool.tile([P, g], F32, tag=f"{tag}_m")
                nc.vector.tensor_reduce(m[:], x3, axis=AX.X, op=ALU.max,
                                        apply_absolute_value=True)
                scale = spool.tile([P, g], F32, tag=f"{tag}_scale")
                nc.vector.tensor_scalar(
                    scale[:].bitcast(I32), m[:].bitcast(I32),
                    0x7F800000, None, op0=ALU.bitwise_and)
                nc.vector.tensor_scalar(
                    scale[:].bitcast(I32), scale[:].bitcast(I32),
                    7 << 23, None, op0=ALU.subtract)
                inv = spool.tile([P, g], F32, tag=f"{tag}_inv")
                nc.vector.tensor_scalar(
                    inv[:].bitcast(I32), scale[:].bitcast(I32),
                    -1, None, op0=ALU.bitwise_xor)
                nc.vector.tensor_scalar(
                    inv[:].bitcast(I32), inv[:].bitcast(I32),
                    (254 << 23) + 1, None, op0=ALU.add)
                return scale, inv

            def q_mult(xt, inv):
                g = in_dim // SZ
                x3 = xt.rearrange("p (g s) -> p g s", s=SZ)
                nc.gpsimd.tensor_tensor(
                    x3, x3, inv[:, :, None].to_broadcast([P, g, SZ]), ALU.mult)

            def q_clamp(xt):
                nc.vector.tensor_scalar(
                    xt, xt, -128.25, 127.25, op0=ALU.max, op1=ALU.min)

            def q_round(xt):
                nc.scalar.activation(xt, xt, AF.Copy, bias=C_RND, scale=1.0)

            def q_stt(xt, scale, tag):
                g = in_dim // SZ
                q = qpool.tile([P, in_dim], BF16, tag=f"{tag}_q")
                nc.vector.scalar_tensor_tensor(
                    q[:].rearrange("p (g s) -> p g s", s=SZ),
                    xt.rearrange("p (g s) -> p g s", s=SZ),
                    C_RND,
                    scale[:, :, None].to_broadcast([P, g, SZ]),
                    op0=ALU.subtract, op1=ALU.mult)
                return q

            def quant_pair(src, rows, dst_dram, tag):
                """Quantize two [P, in_dim] tiles, phase-interleaved, store
                the bf16 results to dst_dram at the same rows."""
                xts = [q_load(src, r, tag) for r in rows]
                si = [q_scales(xt[:], tag) for xt in xts]
                for i, xt in enumerate(xts):
                    q_mult(xt[:], si[i][1][:])
                for xt in xts:
                    q_clamp(xt[:])
                for xt in xts:
                    q_round(xt[:])
                qs = [q_stt(xts[i][:], si[i][0][:], tag)
                      for i in range(len(xts))]
                for i, r in enumerate(rows):
                    nc.scalar.dma_start(dst_dram.ap()[r:r + P, :], qs[i][:])

            # ---- W stage ---------------------------------------------------
            for t0 in range(0, n_wt, 2):
                rows = [r * P for r in range(t0, min(t0 + 2, n_wt))]
                quant_pair(w_sl_t, rows, wq_dram, "q")

            # W transposed reads -> wqT_sb -> wqt_loc (k*P+p row order)
            wqT_sb = big.tile([P, kc, NB], BF16, tag="wqT_sb")
            for k in range(kc):
                nc.scalar.dma_start_transpose(
                    wqT_sb[:, k, :], wq_dram.ap()[:, k * P:(k + 1) * P])
            for kq in range(0, kc, 4):
                nc.sync.dma_start(
                    wqt_loc.ap()[kq * P:(kq + 4) * P, :].rearrange(
                        "(k p) o -> p k o", p=P),
                    wqT_sb[:, kq:kq + 4, :])

            if for_timeline or ncores == 1:
                nc.sync.dma_start(wq_ag.ap()[0:in_dim, :], wqt_loc.ap())
            else:
                nc.gpsimd.collective_compute(
                    "AllGather", ALU.bypass,
                    replica_groups=[list(range(ncores))],
                    ins=[wqt_loc.ap().opt()],
                    outs=[wq_ag.ap().opt()])

            # ---- x half A quant + transposed load --------------------------
            xqT = big.tile([P, kc, b_sh], BF16, tag="xqT")

            def x_half(h):
                for t0 in range(h * nh, (h + 1) * nh, 2):
                    quant_pair(x_sh, [t0 * P, (t0 + 1) * P], xq_dram, "q")
                for k in range(kc):
                    nc.scalar.dma_start_transpose(
                        xqT[:, k, h * bh:(h + 1) * bh],
                        xq_dram.ap()[h * bh:(h + 1) * bh,
                                     k * P:(k + 1) * P])

            x_half(0)

            # ---- PE warmup: keep the tensor engine streaming so it ramps
            # to full clock before (and into) the first real chain ----------
            dummies = []
            if dummy_n:
                for i in range(2):
                    dt_ = mm_pool.tile([P, NB], F32, tag=f"mm{i}",
                                       name=f"dummy_{i}")
                    dummies.append(dt_)
                for i in range(dummy_n):
                    nc.tensor.matmul(
                        dummies[i % 2][:],
                        lhsT=wqT_sb[:, 0, 0:P],
                        rhs=wqT_sb[:, 1, :],
                        start=True, stop=True, skip_group_check=True)
                junk = opool.tile([P, NB], F32, tag="junk")
                for i in range(2):
                    nc.scalar.copy(junk[:], dummies[i][:])

            # ---- matmul passes ---------------------------------------------
            def drain(ps, bb, j):
                s = opool.tile([P, NB], F32, tag="ds")
                nc.vector.tensor_tensor(s[:], ps[:],
                                        b2_sb[:, j * NB:(j + 1) * NB],
                                        ALU.add)
                g = NB // SZ
                s3 = s[:].rearrange("p (g s) -> p g s", s=SZ)
                m = opool.tile([P, g], F32, tag="o_m")
                nc.vector.tensor_reduce(m[:], s3, axis=AX.X, op=ALU.max,
                                        apply_absolute_value=True)
                scale = opool.tile([P, g], F32, tag="o_scale")
                nc.vector.tensor_scalar(
                    scale[:].bitcast(I32), m[:].bitcast(I32),
                    0x7F800000, None, op0=ALU.bitwise_and)
                nc.vector.tensor_scalar(
                    scale[:].bitcast(I32), scale[:].bitcast(I32),
                    7 << 23, None, op0=ALU.subtract)
                inv = opool.tile([P, g], F32, tag="o_inv")
                nc.vector.tensor_scalar(
                    inv[:].bitcast(I32), scale[:].bitcast(I32),
                    -1, None, op0=ALU.bitwise_xor)
                nc.vector.tensor_scalar(
                    inv[:].bitcast(I32), inv[:].bitcast(I32),
                    (254 << 23) + 1, None, op0=ALU.add)
                nc.gpsimd.tensor_tensor(
                    s3, s3, inv[:, :, None].to_broadcast([P, g, SZ]),
                    ALU.mult)
                nc.vector.tensor_scalar(
                    s[:], s[:], -128.25, 127.25, op0=ALU.max, op1=ALU.min)
                nc.scalar.activation(s[:], s[:], AF.Copy, bias=C_RND,
                                     scale=1.0)
                oq = opool.tile([P, NB], F32, tag="oq")
                nc.vector.scalar_tensor_tensor(
                    oq[:].rearrange("p (g s) -> p g s", s=SZ),
                    s[:].rearrange("p (g s) -> p g s", s=SZ),
                    C_RND,
                    scale[:, :, None].to_broadcast([P, g, SZ]),
                    op0=ALU.subtract, op1=ALU.mult)
                nc.scalar.dma_start(
                    out_sh.ap()[bb * P:(bb + 1) * P, j * NB:(j + 1) * NB],
                    oq[:])

            # second-half quant work, emitted piecewise between pass-A blocks
            def _xb_quant(t0):
                def go():
                    quant_pair(x_sh, [t0 * P, (t0 + 1) * P], xq_dram, "q")
                return go

            def _xb_transpose():
                for k in range(kc):
                    nc.scalar.dma_start_transpose(
                        xqT[:, k, bh:2 * bh],
                        xq_dram.ap()[bh:2 * bh, k * P:(k + 1) * P])

            xb_work = [_xb_quant(t0) for t0 in range(nh, n_xt, 2)]
            xb_work.append(_xb_transpose)

            def mm_pass(h):
                bbs = list(range(h * nh, (h + 1) * nh))
                for j in range(n_jb):
                    ps = [mm_pool.tile([P, NB], F32, tag=f"mm{i}",
                                       name=f"ps_{h}_{j}_{i}")
                          for i in range(len(bbs))]
                    for k in range(kc):
                        wqt = wpool.tile([P, NB], BF16, tag="wqt")
                        nc.sync.dma_start(
                            wqt[:],
                            wq_ag.ap()[j * in_dim + k * P:
                                       j * in_dim + (k + 1) * P, :])
                        for i, bb in enumerate(bbs):
                            nc.tensor.matmul(
                                ps[i][:],
                                lhsT=xqT[:, k, bb * P:(bb + 1) * P],
                                rhs=wqt[:],
                                start=(k == 0), stop=(k == kc - 1),
                                skip_group_check=True)
                    for i, bb in enumerate(bbs):
                        drain(ps[i], bb, j)
                    # overlap: second-half quant streams in during pass A
                    if h == 0 and xb_work:
                        xb_work.pop(0)()
                if h == 0:
                    while xb_work:
                        xb_work.pop(0)()

            mm_pass(0)
            mm_pass(1)

    nc.compile()
    return nc


_NC_CACHE = {}


def _get_nc(key=(B_FULL // NCORES, IN_FULL, OUT_FULL, NCORES)):
    if key not in _NC_CACHE:
        _NC_CACHE[key] = build_nc(*key)
    return _NC_CACHE[key]


def make_in_maps(x, W, b, ncores=NCORES):
    b_sh = x.shape[0] // ncores
    w_sl = W.shape[0] // ncores
    out_dim = W.shape[0]
    b2 = (2.0 * np.asarray(b, np.float32)).astype(np.float32)
    b2_rep = np.ascontiguousarray(
        np.broadcast_to(b2.reshape(1, out_dim), (P, out_dim)))
    return [
        {
            "x_sh": np.ascontiguousarray(x[c * b_sh:(c + 1) * b_sh]),
            "w_sl": np.ascontiguousarray(W[c * w_sl:(c + 1) * w_sl]),
            "b2_rep": b2_rep,
        }
        for c in range(ncores)
    ]


def kernel(x, W, b):
    from concourse.bass_utils import run_bass_kernel_spmd

    x = np.asarray(x, np.float32)
    W = np.asarray(W, np.float32)
    b = np.asarray(b, np.float32)
    nc = _get_nc()
    in_maps = make_in_maps(x, W, b)
    res = run_bass_kernel_spmd(nc, in_maps, core_ids=list(range(NCORES)))
    return np.concatenate([res.results[c]["out_sh"] for c in range(NCORES)],
                          axis=0)


# revision 14
# speedup vs baseline: 1.2598x; 1.0948x over previous
"""BFLinear (block-floating-point quantized linear) Trainium2 kernel.

Computes: out = bf_quant(bf_quant(x) @ bf_quant(W).T + 2*b)
where bf_quant quantizes groups of 32 along the last axis to a shared
power-of-two exponent with 8 mantissa bits (values = int8 * 2^(e-7)).

Distribution over 8 NeuronCores:
  - batch dim of x sharded 8 ways (1024 rows/core)
  - W quantization split by output rows (512 rows/core); the quantized
    slab is transposed via XBAR DMA-transpose reads (bit-exact for bf16)
    into [in, out_slice] layout and AllGathered once
  - the transposed quantized x (xqT) is resident in SBUF; all matmul-
    phase weight loads are contiguous 128KB reads
  - matmul runs in bf16 (exact products, fp32 PSUM accumulation; the
    dot products are exactly representable in fp32 for this data)
  - the batch is processed in two half-passes so the second half's
    quantization overlaps the first half's matmuls
  - PE warmup: the tensor engine only reaches 2.4GHz after ~3us of
    gapless execution, so dummy matmuls keep it hot before the first
    real chain and the matmul stream is kept continuous

Quantization math (all exact, matching jnp semantics, verified on HW):
  m     = max |x| over each group of 32          (abs-max reduce)
  scale = 2^(floor(log2 m) - 7)                  (exponent-field bit math)
  inv   = 1/scale                                (bit math, exact)
  r     = rne_round(clamp(x*inv)) via +C trick with C = 1.5*2^23
  q     = (r - C) * scale
"""

import numpy as np

# full-problem dimensions (hardcoded per harness contract)
B_FULL = 8192
IN_FULL = 4096
OUT_FULL = 4096
NCORES = 8

P = 128
SZ = 32
NB = 512  # output column block (= w_sl for 8 cores)
C_RND = float(3 * 2**22)  # 1.5*2^23: v+C stays in [2^23, 2^24) -> RNE to ints


def build_nc(b_sh=B_FULL // NCORES, in_dim=IN_FULL, out_dim=OUT_FULL,
             ncores=NCORES, for_timeline=False, dummy_n=0):
    """Build the SPMD Bass program (identical on every core; data differs)."""
    import concourse.mybir as mybir
    import concourse.tile as tile
    from concourse import bacc

    F32 = mybir.dt.float32
    BF16 = mybir.dt.bfloat16
    I32 = mybir.dt.int32
    ALU = mybir.AluOpType
    AX = mybir.AxisListType
    AF = mybir.ActivationFunctionType

    w_sl = out_dim // ncores          # W rows quantized on this core
    kc = in_dim // P                  # 128-wide contraction chunks
    n_xt = b_sh // P                  # x row tiles
    n_wt = w_sl // P                  # W row tiles
    n_jb = out_dim // NB              # output column blocks (== ncores)
    assert w_sl == NB and n_jb == ncores
    assert in_dim % P == 0 and b_sh % (4 * P) == 0 and n_xt % 4 == 0
    nh = n_xt // 2                    # x tiles per half-pass
    bh = b_sh // 2                    # rows per half-pass

    nc = bacc.Bacc("TRN2", target_bir_lowering=False, debug=False,
                   num_devices=ncores)

    x_sh = nc.dram_tensor("x_sh", [b_sh, in_dim], F32, kind="ExternalInput")
    w_sl_t = nc.dram_tensor("w_sl", [w_sl, in_dim], F32, kind="ExternalInput")
    b2_rep = nc.dram_tensor("b2_rep", [P, out_dim], F32, kind="ExternalInput")
    out_sh = nc.dram_tensor("out_sh", [b_sh, out_dim], F32,
                            kind="ExternalOutput")

    wq_dram = nc.dram_tensor("wq_dram", [w_sl, in_dim], BF16)
    xq_dram = nc.dram_tensor("xq_dram", [b_sh, in_dim], BF16)
    wqt_loc = nc.dram_tensor("wqt_loc", [in_dim, NB], BF16)
    wq_ag = nc.dram_tensor("wq_ag", [ncores * in_dim, NB], BF16,
                           addr_space="Shared")

    with tile.TileContext(nc) as tc:
        from contextlib import ExitStack
        with ExitStack() as ctx:
            qpool = ctx.enter_context(tc.tile_pool(name="qpool", bufs=2))
            spool = ctx.enter_context(tc.tile_pool(name="spool", bufs=2))
            big = ctx.enter_context(tc.tile_pool(name="big", bufs=1))
            wpool = ctx.enter_context(tc.tile_pool(name="wpool", bufs=6))
            opool = ctx.enter_context(tc.tile_pool(name="opool", bufs=4))
            mm_pool = ctx.enter_context(
                tc.tile_pool(name="mmp", bufs=2, space="PSUM"))

            b2_sb = big.tile([P, out_dim], F32, tag="b2_sb")
            nc.scalar.dma_start(b2_sb[:], b2_rep.ap())

            # ---- quant phases (phase-split for software pipelining) --------
            # reduce/bits/clamp/stt are DVE-only ops; the big mult runs on
            # gpsimd (Pool); RNE rounding on the scalar engine.
            def q_load(src, row, tag):
                xt = qpool.tile([P, in_dim], F32, tag=f"{tag}_xt", bufs=4)
                nc.scalar.dma_start(xt[:], src.ap()[row:row + P, :])
                return xt

            def q_scales(xt, tag):
                g = in_dim // SZ
                x3 = xt.rearrange("p (g s) -> p g s", s=SZ)
                m = spool.tile([P, g], F32, tag=f"{tag}_m")
                nc.vector.tensor_reduce(m[:], x3, axis=AX.X, op=ALU.max,
                                        apply_absolute_value=True)
                scale = spool.tile([P, g], F32, tag=f"{tag}_scale")
                nc.vector.tensor_scalar(
                    scale[:].bitcast(I32), m[:].bitcast(I32),
                    0x7F800000, None, op0=ALU.bitwise_and)
                nc.vector.tensor_scalar(
                    scale[:].bitcast(I32), scale[:].bitcast(I32),
                    7 << 23, None, op0=ALU.subtract)
                inv = spool.tile([P, g], F32, tag=f"{tag}_inv")
                nc.vector.tensor_scalar(
                    inv[:].bitcast(I32), scale[:].bitcast(I32),
                    -1, None, op0=ALU.bitwise_xor)
                nc.vector.tensor_scalar(
                    inv[:].bitcast(I32), inv[:].bitcast(I32),
                    (254 << 23) + 1, None, op0=ALU.add)
                return scale, inv

            def q_mult(xt, inv):
                g = in_dim // SZ
                x3 = xt.rearrange("p (g s) -> p g s", s=SZ)
                nc.gpsimd.tensor_tensor(
                    x3, x3, inv[:, :, None].to_broadcast([P, g, SZ]), ALU.mult)

            def q_clamp(xt):
                nc.vector.tensor_scalar(
                    xt, xt, -128.25, 127.25, op0=ALU.max, op1=ALU.min)

            def q_round(xt):
                nc.scalar.activation(xt, xt, AF.Copy, bias=C_RND, scale=1.0)

            def q_stt(xt, scale, tag):
                g = in_dim // SZ
                q = qpool.tile([P, in_dim], BF16, tag=f"{tag}_q")
                nc.vector.scalar_tensor_tensor(
                    q[:].rearrange("p (g s) -> p g s", s=SZ),
                    xt.rearrange("p (g s) -> p g s", s=SZ),
                    C_RND,
                    scale[:, :, None].to_broadcast([P, g, SZ]),
                    op0=ALU.subtract, op1=ALU.mult)
                return q

            def quant_pair(src, rows, dst_dram, tag):
                """Quantize two [P, in_dim] tiles, phase-interleaved, store
                the bf16 results to dst_dram at the same rows."""
                xts = [q_load(src, r, tag) for r in rows]
                si = [q_scales(xt[:], tag) for xt in xts]
                for i, xt in enumerate(xts):
                    q_mult(xt[:], si[i][1][:])
                for xt in xts:
                    q_clamp(xt[:])
                for xt in xts:
                    q_round(xt[:])
                qs = [q_stt(xts[i][:], si[i][0][:], tag)
                      for i in range(len(xts))]
                for i, r in enumerate(rows):
                    nc.scalar.dma_start(dst_dram.ap()[r:r + P, :], qs[i][:])

            # ---- W stage ---------------------------------------------------
            for t0 in range(0, n_wt, 2):
                rows = [r * P for r in range(t0, min(t0 + 2, n_wt))]
                quant_pair(w_sl_t, rows, wq_dram, "q")

            # W transposed reads -> small rotating buffer -> wqt_loc
            for kq in range(0, kc, 4):
                wtt = qpool.tile([P, 4, NB], BF16, tag="wtt", bufs=2)
                for t in range(4):
                    k = kq + t
                    nc.scalar.dma_start_transpose(
                        wtt[:, t, :], wq_dram.ap()[:, k * P:(k + 1) * P])
                nc.sync.dma_start(
                    wqt_loc.ap()[kq * P:(kq + 4) * P, :].rearrange(
                        "(k p) o -> p k o", p=P),
                    wtt[:])

            if for_timeline or ncores == 1:
                nc.sync.dma_start(wq_ag.ap()[0:in_dim, :], wqt_loc.ap())
            else:
                nc.gpsimd.collective_compute(
                    "AllGather", ALU.bypass,
                    replica_groups=[list(range(ncores))],
                    ins=[wqt_loc.ap().opt()],
                    outs=[wq_ag.ap().opt()])

            # ---- x half A quant + transposed load --------------------------
            xqT = big.tile([P, kc, b_sh], BF16, tag="xqT")

            def x_half(h):
                for t0 in range(h * nh, (h + 1) * nh, 2):
                    quant_pair(x_sh, [t0 * P, (t0 + 1) * P], xq_dram, "q")
                for k in range(kc):
                    nc.scalar.dma_start_transpose(
                        xqT[:, k, h * bh:(h + 1) * bh],
                        xq_dram.ap()[h * bh:(h + 1) * bh,
                                     k * P:(k + 1) * P])

            x_half(0)

            # ---- matmul passes ---------------------------------------------
            def drain(ps, bb, j):
                s = opool.tile([P, NB], F32, tag="ds")
                nc.vector.tensor_tensor(s[:], ps[:],
                                        b2_sb[:, j * NB:(j + 1) * NB],
                                        ALU.add)
                g = NB // SZ
                s3 = s[:].rearrange("p (g s) -> p g s", s=SZ)
                m = opool.tile([P, g], F32, tag="o_m")
                nc.vector.tensor_reduce(m[:], s3, axis=AX.X, op=ALU.max,
                                        apply_absolute_value=True)
                scale = opool.tile([P, g], F32, tag="o_scale")
                nc.vector.tensor_scalar(
                    scale[:].bitcast(I32), m[:].bitcast(I32),
                    0x7F800000, None, op0=ALU.bitwise_and)
                nc.vector.tensor_scalar(
                    scale[:].bitcast(I32), scale[:].bitcast(I32),
                    7 << 23, None, op0=ALU.subtract)
                inv = opool.tile([P, g], F32, tag="o_inv")
                nc.vector.tensor_scalar(
                    inv[:].bitcast(I32), scale[:].bitcast(I32),
                    -1, None, op0=ALU.bitwise_xor)
                nc.vector.tensor_scalar(
                    inv[:].bitcast(I32), inv[:].bitcast(I32),
                    (254 << 23) + 1, None, op0=ALU.add)
                nc.gpsimd.tensor_tensor(
                    s3, s3, inv[:, :, None].to_broadcast([P, g, SZ]),
                    ALU.mult)
                nc.vector.tensor_scalar(
                    s[:], s[:], -128.25, 127.25, op0=ALU.max, op1=ALU.min)
                nc.scalar.activation(s[:], s[:], AF.Copy, bias=C_RND,
                                     scale=1.0)
                oq = opool.tile([P, NB], F32, tag="oq")
                nc.vector.scalar_tensor_tensor(
                    oq[:].rearrange("p (g s) -> p g s", s=SZ),
                    s[:].rearrange("p (g s) -> p g s", s=SZ),
                    C_RND,
                    scale[:, :, None].to_broadcast([P, g, SZ]),
                    op0=ALU.subtract, op1=ALU.mult)
                nc.scalar.dma_start(
                    out_sh.ap()[bb * P:(bb + 1) * P, j * NB:(j + 1) * NB],
                    oq[:])

            # second-half quant work, emitted piecewise between pass-A blocks
            def _xb_quant(t0):
                def go():
                    quant_pair(x_sh, [t0 * P, (t0 + 1) * P], xq_dram, "q")
                return go

            def _xb_transpose():
                for k in range(kc):
                    nc.scalar.dma_start_transpose(
                        xqT[:, k, bh:2 * bh],
                        xq_dram.ap()[bh:2 * bh, k * P:(k + 1) * P])

            xb_work = [_xb_quant(t0) for t0 in range(nh, n_xt, 2)]
            xb_work.append(_xb_transpose)

            def mm_pass(h):
                bbs = list(range(h * nh, (h + 1) * nh))
                for j in range(n_jb):
                    ps = [mm_pool.tile([P, NB], F32, tag=f"mm{i}",
                                       name=f"ps_{h}_{j}_{i}")
                          for i in range(len(bbs))]
                    for k in range(kc):
                        wqt = wpool.tile([P, NB], BF16, tag="wqt")
                        nc.sync.dma_start(
                            wqt[:],
                            wq_ag.ap()[j * in_dim + k * P:
                                       j * in_dim + (k + 1) * P, :])
                        for i, bb in enumerate(bbs):
                            nc.tensor.matmul(
                                ps[i][:],
                                lhsT=xqT[:, k, bb * P:(bb + 1) * P],
                                rhs=wqt[:],
                                start=(k == 0), stop=(k == kc - 1),
                                skip_group_check=True)
                    for i, bb in enumerate(bbs):
                        drain(ps[i], bb, j)
                    # overlap: second-half quant streams in during pass A
                    if h == 0 and xb_work:
                        xb_work.pop(0)()
                if h == 0:
                    while xb_work:
                        xb_work.pop(0)()

            mm_pass(0)
            mm_pass(1)

    nc.compile()
    return nc


_NC_CACHE = {}


def _get_nc(key=(B_FULL // NCORES, IN_FULL, OUT_FULL, NCORES)):
    if key not in _NC_CACHE:
        _NC_CACHE[key] = build_nc(*key)
    return _NC_CACHE[key]


def make_in_maps(x, W, b, ncores=NCORES):
    b_sh = x.shape[0] // ncores
    w_sl = W.shape[0] // ncores
    out_dim = W.shape[0]
    b2 = (2.0 * np.asarray(b, np.float32)).astype(np.float32)
    b2_rep = np.ascontiguousarray(
        np.broadcast_to(b2.reshape(1, out_dim), (P, out_dim)))
    return [
        {
            "x_sh": np.ascontiguousarray(x[c * b_sh:(c + 1) * b_sh]),
            "w_sl": np.ascontiguousarray(W[c * w_sl:(c + 1) * w_sl]),
            "b2_rep": b2_rep,
        }
        for c in range(ncores)
    ]


def kernel(x, W, b):
    from concourse.bass_utils import run_bass_kernel_spmd

    x = np.asarray(x, np.float32)
    W = np.asarray(W, np.float32)
    b = np.asarray(b, np.float32)
    nc = _get_nc()
    in_maps = make_in_maps(x, W, b)
    res = run_bass_kernel_spmd(nc, in_maps, core_ids=list(range(NCORES)))
    return np.concatenate([res.results[c]["out_sh"] for c in range(NCORES)],
                          axis=0)
